# revision 12
# baseline (speedup 1.0000x reference)
"""BestRQ loss kernel for 8 Trainium2 NeuronCores.

Sharding: data-parallel over batch B=8 (one batch element per core, no
collectives). Each core runs: embed -> codebook argmin (targets) ->
mask-fill -> 6-layer transformer encoder -> logits over V=8192 ->
log-softmax gather -> partial masked sum. Host divides by mask count.

Device layout: residual stream is kept transposed [D, T] with D on SBUF
partitions (d-major), so every weight matmul uses weights in their natural
[din, dout] layout as lhsT. Attention scores are computed kpos-major so the
padding bias is a per-partition scalar folded into the ACT exp. V carries an
appended ones-column so the softmax denominators fall out of the AV matmul.
"""

import contextlib

import numpy as np
import ml_dtypes

import concourse.bass as bass
import concourse.tile as tile
from concourse import mybir
from concourse.bass_utils import run_bass_kernel_spmd
from concourse.masks import make_identity

F32 = mybir.dt.float32
BF16 = mybir.dt.bfloat16
AX = mybir.AxisListType
OP = mybir.AluOpType
AF = mybir.ActivationFunctionType

D = 512
T = 1024
V = 8192
NCB = 1
L = 6
H = 8
HD = 64
F = 2048
P = 128
KT = D // P       # 4 k-tiles over D
TT = T // P       # 8 token tiles
VC = V // 512     # 16 v chunks
EPS = 1e-5
NEG = -10000.0    # pad bias: exp(x*0.125 + NEG) == 0.0 exactly in fp32


def _legalize_single_wait(nc):
    """walrus in this container supports one sync-wait per instruction;
    split Tile's multi-wait tail drain into single-wait NOPs."""
    ctr = 0
    for fn in nc.m.functions:
        for bb in fn.blocks:
            insts = list(bb.instructions)
            out = []
            changed = False
            for inst in insts:
                si = getattr(inst, "sync_info", None)
                if si is not None and len(si.on_wait) > 1:
                    changed = True
                    waits = list(si.on_wait)
                    for w in waits[:-1]:
                        ctr += 1
                        nop = mybir.InstNoOp(name=f"{inst.name}-sw{ctr}", ins=[], outs=[])
                        nop.engine = inst.engine
                        nop.sync_info = mybir.SyncInfo(on_wait=[w], on_update=[])
                        out.append(nop)
                    inst.sync_info = mybir.SyncInfo(
                        on_wait=[waits[-1]], on_update=list(si.on_update)
                    )
                out.append(inst)
            if changed:
                bb.instructions = out
    return ctr


def _dram(nc, name, shape, dtype, out=False):
    return nc.declare_dram_parameter(name, list(shape), dtype, isOutput=out)


def _standardize(nc, sbuf, rows, ps_stat, r, ones_t, sq_tag):
    """Per-token standardize of d-major tile r [128, KT, T] (fp32).
    Returns (rstd, negmrs) [128, T] fp32 partition-replicated tiles."""
    sq = sbuf.tile([P, KT, T], F32, tag=sq_tag, name="lnsq")
    nc.scalar.square(sq[:], r[:])
    mean = rows.tile([P, T], F32, tag="lnrow", name="mean")
    var = rows.tile([P, T], F32, tag="lnrow", name="var")
    for ch in range(2):
        sl = bass.ts(ch, 512)
        sum_ps = ps_stat.tile([P, 512], F32, tag="ps_stat", name="sum_ps")
        for k in range(KT):
            nc.tensor.matmul(sum_ps[:], ones_t[:], r[:, k, sl],
                             start=(k == 0), stop=(k == KT - 1))
        nc.vector.tensor_scalar_mul(mean[:, sl], sum_ps[:], 1.0 / D)
        ssq_ps = ps_stat.tile([P, 512], F32, tag="ps_stat", name="ssq_ps")
        for k in range(KT):
            nc.tensor.matmul(ssq_ps[:], ones_t[:], sq[:, k, sl],
                             start=(k == 0), stop=(k == KT - 1))
        nc.vector.tensor_scalar_mul(var[:, sl], ssq_ps[:], 1.0 / D)
    # var = ssq/D - mean^2 + eps ; rstd = 1/sqrt(var) ; negmrs = -mean*rstd
    sd = rows.tile([P, T], F32, tag="lnrow", name="sd")
    nc.vector.tensor_tensor(sd[:], mean[:], mean[:], OP.mult)
    nc.vector.tensor_tensor(var[:], var[:], sd[:], OP.subtract)
    nc.vector.tensor_scalar_add(var[:], var[:], float(EPS))
    nc.scalar.activation(sd[:], var[:], AF.Sqrt)
    rstd = rows.tile([P, T], F32, tag="lnrow", name="rstd")
    nc.vector.reciprocal(rstd[:], sd[:])
    nc.vector.tensor_tensor(mean[:], mean[:], rstd[:], OP.mult)
    nc.vector.tensor_scalar_mul(mean[:], mean[:], -1.0)
    return rstd, mean


def _ln_to(nc, sbuf, rows, ps_stat, r, ones_t, s_t, b_t, out_tile, sq_tag, tmp_tag):
    """out_tile[:, k] = standardize(r)*s + b  (per-ko scale/bias), any out dtype."""
    rstd, negmrs = _standardize(nc, sbuf, rows, ps_stat, r, ones_t, sq_tag)
    u = sbuf.tile([P, KT, T], F32, tag=tmp_tag, name="lntmp")
    for k in range(KT):
        nc.vector.tensor_tensor(u[:, k], r[:, k], rstd[:], OP.mult)
        nc.vector.tensor_tensor(u[:, k], u[:, k], negmrs[:], OP.add)
        nc.vector.tensor_scalar(out_tile[:, k], u[:, k], s_t[:, k : k + 1],
                                b_t[:, k : k + 1], OP.mult, OP.add)


def build_nc():
    nc = bass.Bass()

    # ---- per-core inputs (d-major / natural weight layouts) ----
    xsT_d = _dram(nc, "xsT", [D, T], F32)
    peT_d = _dram(nc, "peT", [D, T], F32)
    wemb_d = _dram(nc, "wemb", [D, D], F32)
    proj_d = _dram(nc, "proj", [D, D], F32)
    emb_d = _dram(nc, "emb", [D, V], F32)
    hesq_d = _dram(nc, "hesq", [P, V], F32)
    ilns_d = _dram(nc, "ilns", [D], F32)
    ilnb_d = _dram(nc, "ilnb", [D], F32)
    memb_d = _dram(nc, "memb", [D], F32)
    mask01_d = _dram(nc, "mask01", [P, T], F32)
    notmask_d = _dram(nc, "notmask", [P, T], F32)
    padb_d = _dram(nc, "padb", [T], F32)
    mval_d = _dram(nc, "mval", [T], F32)
    wqkv_d = _dram(nc, "wqkv", [L, D, 3 * D], BF16)
    wo_d = _dram(nc, "wo", [L, D, D], BF16)
    w1_d = _dram(nc, "w1", [L, D, F], BF16)
    w2_d = _dram(nc, "w2", [L, F, D], BF16)
    ln1s_d = _dram(nc, "ln1s", [P, KT, L], F32)
    ln1b_d = _dram(nc, "ln1b", [P, KT, L], F32)
    ln2s_d = _dram(nc, "ln2s", [P, KT, L], F32)
    ln2b_d = _dram(nc, "ln2b", [P, KT, L], F32)
    ans_d = _dram(nc, "ans", [D], F32)
    anb_d = _dram(nc, "anb", [D], F32)
    top_d = _dram(nc, "top", [D, V], BF16)
    topT_d = _dram(nc, "topT", [V, D], F32)

    num_d = _dram(nc, "num", [1, 1], F32, out=True)

    kp = lambda ap: ap.rearrange("(ko p) -> p ko", p=P)       # [D] -> [128, KT]
    dm = lambda ap: ap.rearrange("(ko p) t -> p ko t", p=P)   # [D,T] -> [128,KT,T]

    stack = contextlib.ExitStack()
    with tile.TileContext(nc) as tc, stack:
        persist = stack.enter_context(tc.tile_pool(name="persist", bufs=1))
        rows = stack.enter_context(tc.tile_pool(name="rows", bufs=4))
        tiny = stack.enter_context(tc.tile_pool(name="tiny", bufs=2))
        ps_stat = stack.enter_context(tc.tile_pool(name="ps_stat", bufs=2, space="PSUM"))
        ps_mm = stack.enter_context(tc.tile_pool(name="ps_mm", bufs=4, space="PSUM"))

        ones_t = persist.tile([P, P], F32)
        nc.vector.memset(ones_t[:], 1.0)
        ident = persist.tile([P, P], F32)
        make_identity(nc, ident[:])
        r_t = persist.tile([P, KT, T], F32)          # residual stream (masked)
        tgt_t = persist.tile([P, TT], mybir.dt.int32)
        tlogit_t = persist.tile([P, TT], F32)
        lse_t = persist.tile([P, TT], F32)
        padb_t = persist.tile([P, TT], F32)
        nc.sync.dma_start(out=padb_t[:], in_=padb_d.rearrange("(tt p) -> p tt", p=P))
        mval_t = persist.tile([P, TT], F32)
        nc.sync.dma_start(out=mval_t[:], in_=mval_d.rearrange("(tt p) -> p tt", p=P))

        # ================= stage A/B: embed, targets =================
        with tc.tile_pool(name="pre", bufs=1) as pre:
            x_t = pre.tile([P, KT, T], F32)
            ui_t = pre.tile([P, KT, T], F32)
            z_t = pre.tile([P, KT, T], F32)
            with tc.tile_pool(name="preA", bufs=1) as preA:
                xsT_t = preA.tile([P, KT, T], F32)
                nc.sync.dma_start(out=xsT_t[:], in_=dm(xsT_d))
                peT_t = preA.tile([P, KT, T], F32)
                nc.sync.dma_start(out=peT_t[:], in_=dm(peT_d))
                wemb_t = preA.tile([P, KT, D], F32)
                nc.sync.dma_start(out=wemb_t[:], in_=dm(wemb_d))
                for mo in range(KT):
                    for ch in range(2):
                        sl = bass.ts(ch, 512)
                        ps = ps_mm.tile([P, 512], F32, tag="ps", name="ps")
                        for k in range(KT):
                            nc.tensor.matmul(ps[:], wemb_t[:, k, bass.ts(mo, P)],
                                             xsT_t[:, k, sl],
                                             start=(k == 0), stop=(k == KT - 1))
                        nc.vector.tensor_tensor(x_t[:, mo, sl], ps[:], peT_t[:, mo, sl], OP.add)

                # masked residual r = x*notmask + mask_emb*mask (encoder input)
                mask01_t = preA.tile([P, T], F32)
                nc.sync.dma_start(out=mask01_t[:], in_=mask01_d[:])
                notmask_t = preA.tile([P, T], F32)
                nc.sync.dma_start(out=notmask_t[:], in_=notmask_d[:])
                memb_t = preA.tile([P, KT], F32)
                nc.sync.dma_start(out=memb_t[:], in_=kp(memb_d))
                for k in range(KT):
                    nc.vector.tensor_tensor(r_t[:, k], x_t[:, k], notmask_t[:], OP.mult)
                    nc.vector.scalar_tensor_tensor(
                        r_t[:, k], mask01_t[:],
                        memb_t[:, k : k + 1], r_t[:, k], OP.mult, OP.add)

                # iln -> ui
                ilns_t = preA.tile([P, KT], F32)
                nc.sync.dma_start(out=ilns_t[:], in_=kp(ilns_d))
                ilnb_t = preA.tile([P, KT], F32)
                nc.sync.dma_start(out=ilnb_t[:], in_=kp(ilnb_d))
                _ln_to(nc, preA, rows, ps_stat, x_t, ones_t, ilns_t, ilnb_t, ui_t,
                       "lnscr_pre", "lnscr_pre")

                # projection -> z (fp32)
                proj_t = preA.tile([P, KT, D], F32)
                nc.sync.dma_start(out=proj_t[:], in_=dm(proj_d))
                for mo in range(KT):
                    for ch in range(2):
                        sl = bass.ts(ch, 512)
                        ps = ps_mm.tile([P, 512], F32, tag="ps", name="ps")
                        for k in range(KT):
                            nc.tensor.matmul(ps[:], proj_t[:, k, bass.ts(mo, P)],
                                             ui_t[:, k, sl],
                                             start=(k == 0), stop=(k == KT - 1))
                        nc.vector.tensor_copy(z_t[:, mo, sl], ps[:])

            # cdist scores + argmax, token tiles in groups of 2
            with tc.tile_pool(name="cd", bufs=1) as cd, \
                 tc.tile_pool(name="cd2", bufs=2) as cd2:
                hesq_t = cd.tile([P, V], F32)
                nc.sync.dma_start(out=hesq_t[:], in_=hesq_d[:])
                for g in range(4):
                    sc_tiles = []
                    for i in range(2):
                        sc_tiles.append(cd.tile([P, V], F32, tag=f"sc{i}", name=f"sc{i}"))
                    for vc in range(VC):
                        vsl = bass.ts(vc, 512)
                        e_t = cd2.tile([P, KT, 512], F32, tag="echunk", name="echunk")
                        nc.sync.dma_start(out=e_t[:], in_=dm(emb_d)[:, :, vsl])
                        for i in range(2):
                            tt = g * 2 + i
                            ps = ps_mm.tile([P, 512], F32, tag="ps", name="ps")
                            for k in range(KT):
                                nc.tensor.matmul(ps[:], z_t[:, k, bass.ts(tt, P)],
                                                 e_t[:, k],
                                                 start=(k == 0), stop=(k == KT - 1))
                            nc.vector.tensor_tensor(
                                sc_tiles[i][:, vsl], ps[:],
                                hesq_t[:, vsl], OP.subtract)
                    for i in range(2):
                        tt = g * 2 + i
                        mv = tiny.tile([P, 8], F32, tag="mv", name="mv")
                        mi = tiny.tile([P, 8], mybir.dt.uint32, tag="mi", name="mi")
                        nc.vector.max(mv[:], sc_tiles[i][:])
                        nc.vector.max_index(mi[:], mv[:], sc_tiles[i][:])
                        nc.vector.tensor_copy(tgt_t[:, tt : tt + 1], mi[:, 0:1])

        # ================= stage D: encoder =================
        enc_stack = contextlib.ExitStack()
        enc = enc_stack.enter_context(tc.tile_pool(name="enc", bufs=1))
        encw = enc_stack.enter_context(tc.tile_pool(name="encw", bufs=1))
        encw1 = enc_stack.enter_context(tc.tile_pool(name="encw1", bufs=1))
        ps_av = enc_stack.enter_context(tc.tile_pool(name="ps_av", bufs=2, space="PSUM"))

        ln1s_t = enc.tile([P, KT, L], F32)
        nc.sync.dma_start(out=ln1s_t[:], in_=ln1s_d[:])
        ln1b_t = enc.tile([P, KT, L], F32)
        nc.sync.dma_start(out=ln1b_t[:], in_=ln1b_d[:])
        ln2s_t = enc.tile([P, KT, L], F32)
        nc.sync.dma_start(out=ln2s_t[:], in_=ln2s_d[:])
        ln2b_t = enc.tile([P, KT, L], F32)
        nc.sync.dma_start(out=ln2b_t[:], in_=ln2b_d[:])

        for l in range(L):
            wqkv_t = encw.tile([P, KT, 3 * D], BF16, tag="wqkv", name="wqkv")
            nc.sync.dma_start(out=wqkv_t[:],
                              in_=wqkv_d[l].rearrange("(ko p) m -> p ko m", p=P))
            # LN1 -> u (bf16)
            u_t = enc.tile([P, KT, T], BF16, tag="u", name="u")
            _ln_to(nc, enc, rows, ps_stat, r_t, ones_t,
                   ln1s_t[:, :, l], ln1b_t[:, :, l], u_t, "lnscr", "lnscr")

            # q,k dims-major: qk [128, 8mo, T] bf16
            qk_t = enc.tile([P, 8, T], BF16, tag="qk", name="qk")
            for mo in range(8):
                for ch in range(2):
                    sl = bass.ts(ch, 512)
                    ps = ps_mm.tile([P, 512], F32, tag="ps", name="ps")
                    for k in range(KT):
                        nc.tensor.matmul(ps[:], wqkv_t[:, k, bass.ts(mo, P)],
                                         u_t[:, k, sl],
                                         start=(k == 0), stop=(k == KT - 1))
                    nc.vector.tensor_copy(qk_t[:, mo, sl], ps[:])

            # v token-major with appended ones column: v [128, tt, h, 65]
            v_t = enc.tile([P, TT, H, 2 * HD], BF16, tag="v", name="v")
            nc.vector.memset(v_t[:, :, :, HD : 2 * HD], 1.0)
            for tt in range(TT):
                ps = ps_mm.tile([P, 512], F32, tag="ps", name="ps")
                for k in range(KT):
                    nc.tensor.matmul(ps[:], u_t[:, k, bass.ts(tt, P)],
                                     wqkv_t[:, k, 2 * D : 3 * D],
                                     start=(k == 0), stop=(k == KT - 1))
                nc.vector.tensor_copy(
                    v_t[:, tt, :, 0:HD],
                    ps[:].rearrange("p (h d) -> p h d", h=H))

            # attention, head by head; attnT [128, KT(hd-major), T] bf16
            attnT_t = enc.tile([P, KT, T], BF16, tag="attnT", name="attnT")
            for h in range(H):
                bp = 64 * (h % 2)
                qmo, kmo = h // 2, 4 + h // 2
                a_t = enc.tile([P, TT, T], BF16, tag="a", name="a")
                for kt in range(TT):
                    for ch in range(2):
                        sl = bass.ts(ch, 512)
                        ps = ps_mm.tile([P, 512], F32, tag="ps", name="ps")
                        nc.tensor.matmul(
                            ps[:], qk_t[bp : bp + HD, kmo, bass.ts(kt, P)],
                            qk_t[bp : bp + HD, qmo, sl], start=True, stop=True)
                        nc.scalar.activation(a_t[:, kt, sl], ps[:], AF.Exp,
                                             bias=padb_t[:, kt : kt + 1], scale=0.125)
                for ch in range(2):
                    sl = bass.ts(ch, 512)
                    pso = ps_av.tile([P, 512], F32, tag="ps_av", name="ps_av")
                    for kt in range(TT):
                        nc.tensor.matmul(pso[:], v_t[:, kt, h, :], a_t[:, kt, sl],
                                         start=(kt == 0), stop=(kt == TT - 1))
                    rec = tiny.tile([HD, 512], F32, tag="rec", name="rec")
                    nc.vector.reciprocal(rec[:], pso[HD : 2 * HD, :])
                    nc.vector.tensor_tensor(
                        attnT_t[bp : bp + HD, h // 2, sl], pso[0:HD, :],
                        rec[:], OP.mult)

            # Wo + residual
            wo_t = encw1.tile([P, KT, D], BF16, tag="wo", name="wo")
            nc.sync.dma_start(out=wo_t[:], in_=wo_d[l].rearrange("(ko p) m -> p ko m", p=P))
            for mo in range(KT):
                for ch in range(2):
                    sl = bass.ts(ch, 512)
                    ps = ps_mm.tile([P, 512], F32, tag="ps", name="ps")
                    for k in range(KT):
                        nc.tensor.matmul(ps[:], wo_t[:, k, bass.ts(mo, P)],
                                         attnT_t[:, k, sl],
                                         start=(k == 0), stop=(k == KT - 1))
                    nc.vector.tensor_tensor(r_t[:, mo, sl], r_t[:, mo, sl], ps[:], OP.add)

            # LN2 -> u2 (bf16), FFN
            u2_t = enc.tile([P, KT, T], BF16, tag="u", name="u2")
            _ln_to(nc, enc, rows, ps_stat, r_t, ones_t,
                   ln2s_t[:, :, l], ln2b_t[:, :, l], u2_t, "lnscr", "lnscr")
            w1_t = encw1.tile([P, KT, F], BF16, tag="w1", name="w1")
            nc.sync.dma_start(out=w1_t[:], in_=w1_d[l].rearrange("(ko p) m -> p ko m", p=P))
            f_t = enc.tile([P, F // P, T], BF16, tag="f", name="f")
            for mo in range(F // P):
                for ch in range(2):
                    sl = bass.ts(ch, 512)
                    ps = ps_mm.tile([P, 512], F32, tag="ps", name="ps")
                    for k in range(KT):
                        nc.tensor.matmul(ps[:], w1_t[:, k, bass.ts(mo, P)],
                                         u2_t[:, k, sl],
                                         start=(k == 0), stop=(k == KT - 1))
                    nc.scalar.activation(f_t[:, mo, sl], ps[:], AF.Relu)
            w2_t = encw1.tile([P, F // P, D], BF16, tag="w2", name="w2")
            nc.sync.dma_start(out=w2_t[:], in_=w2_d[l].rearrange("(ko p) m -> p ko m", p=P))
            for mo in range(KT):
                for ch in range(2):
                    sl = bass.ts(ch, 512)
                    ps = ps_mm.tile([P, 512], F32, tag="ps", name="ps")
                    for k in range(F // P):
                        nc.tensor.matmul(ps[:], w2_t[:, k, bass.ts(mo, P)],
                                         f_t[:, k, sl],
                                         start=(k == 0), stop=(k == F // P - 1))
                    nc.vector.tensor_tensor(r_t[:, mo, sl], r_t[:, mo, sl], ps[:], OP.add)

        enc_stack.close()

        # ================= stage E: final LN, logits, loss =================
        with tc.tile_pool(name="fin", bufs=1) as fin, \
             tc.tile_pool(name="fin2", bufs=3) as fin2:
            ans_t = fin.tile([P, KT], F32)
            nc.sync.dma_start(out=ans_t[:], in_=kp(ans_d))
            anb_t = fin.tile([P, KT], F32)
            nc.sync.dma_start(out=anb_t[:], in_=kp(anb_d))
            uan_t = fin.tile([P, KT, T], F32)
            _ln_to(nc, fin, rows, ps_stat, r_t, ones_t, ans_t, anb_t, uan_t,
                   "lnscr_fin", "lnscr_fin")
            uanb_t = fin.tile([P, KT, T], BF16)
            nc.vector.tensor_copy(uanb_t[:], uan_t[:])

            # gather target rows of topT, dot with u_an (token-major transpose)
            for tt in range(TT):
                utok = fin2.tile([P, D], F32, tag="utok", name="utok")
                for k in range(KT):
                    pst = ps_mm.tile([P, 512], F32, tag="ps", name="pst")
                    nc.tensor.transpose(pst[:, 0:P], uan_t[:, k, bass.ts(tt, P)], ident[:])
                    nc.vector.tensor_copy(utok[:, bass.ts(k, P)], pst[:, 0:P])
                g_t = fin2.tile([P, D], F32, tag="g", name="g")
                nc.gpsimd.indirect_dma_start(
                    out=g_t[:], out_offset=None, in_=topT_d[:],
                    in_offset=bass.IndirectOffsetOnAxis(ap=tgt_t[:, tt : tt + 1], axis=0))
                prod = fin2.tile([P, D], F32, tag="prod", name="prod")
                nc.vector.tensor_tensor(prod[:], g_t[:], utok[:], OP.mult)
                nc.vector.reduce_sum(tlogit_t[:, tt : tt + 1], prod[:], axis=AX.X)

            # logits + lse
            acc_t = fin.tile([P, TT, VC], F32)
            for vc in range(VC):
                vsl = bass.ts(vc, 512)
                top_t = fin2.tile([P, KT, 512], BF16, tag="topchunk", name="topchunk")
                nc.sync.dma_start(out=top_t[:], in_=dm(top_d)[:, :, vsl])
                for tt in range(TT):
                    ps = ps_mm.tile([P, 512], F32, tag="ps", name="ps")
                    for k in range(KT):
                        nc.tensor.matmul(ps[:], uanb_t[:, k, bass.ts(tt, P)],
                                         top_t[:, k],
                                         start=(k == 0), stop=(k == KT - 1))
                    esc = fin2.tile([P, 512], BF16, tag="esc", name="esc")
                    nc.scalar.activation(esc[:], ps[:], AF.Exp,
                                         accum_out=acc_t[:, tt, vc : vc + 1])
            sume = fin.tile([P, TT], F32)
            nc.vector.reduce_sum(sume[:], acc_t[:], axis=AX.X)
            nc.scalar.activation(lse_t[:], sume[:], AF.Ln)

            # num = sum((lse - tlogit) * mval)
            ent = fin.tile([P, TT], F32)
            nc.vector.tensor_tensor(ent[:], lse_t[:], tlogit_t[:], OP.subtract)
            nc.vector.tensor_tensor(ent[:], ent[:], mval_t[:], OP.mult)
            entsum = fin.tile([P, 1], F32)
            nc.vector.reduce_sum(entsum[:], ent[:], axis=AX.X)
            nps = ps_stat.tile([1, 1], F32, tag="ps_stat", name="nps")
            nc.tensor.matmul(nps[:], ones_t[:, 0:1], entsum[:], start=True, stop=True)
            nout = fin.tile([1, 1], F32)
            nc.vector.tensor_copy(nout[:], nps[:])
            nc.sync.dma_start(out=num_d[:], in_=nout[:])

    _legalize_single_wait(nc)
    return nc


def make_inputs(xs, xs_lens, mask_indices, W_embed, ln1_s, ln1_b, Wqkv, Wo,
                ln2_s, ln2_b, W1, W2, an_s, an_b, iln_s, iln_b, mask_emb,
                top_n_out, projection, embeddings):
    """Host-side prep: transposes, casts, masks, PE table. Returns in_maps."""
    B = xs.shape[0]
    bf = lambda a: np.ascontiguousarray(a).astype(ml_dtypes.bfloat16)
    lnp = lambda a: np.ascontiguousarray(
        np.transpose(np.asarray(a, np.float32).reshape(L, KT, P), (2, 1, 0)))
    f32 = lambda a: np.ascontiguousarray(np.asarray(a, np.float32))

    pos = np.arange(T, dtype=np.float32)[:, None]
    i = np.arange(0, D, 2, dtype=np.float32)[None, :]
    ang = pos / np.power(10000.0, i / D)
    pe = np.zeros((T, D), np.float32)
    pe[:, 0::2] = np.sin(ang)
    pe[:, 1::2] = np.cos(ang)

    emb = np.asarray(embeddings[0], np.float32)          # [E, V]
    hesq = 0.5 * (emb.astype(np.float64) ** 2).sum(0).astype(np.float32)[None, :]
    top = np.asarray(top_n_out[0], np.float32)           # [D, V]

    shared = {
        "peT": f32(pe.T),
        "wemb": f32(W_embed),
        "proj": f32(projection),
        "emb": f32(emb),
        "hesq": f32(np.broadcast_to(hesq, (P, V))),
        "ilns": f32(iln_s), "ilnb": f32(iln_b),
        "memb": f32(mask_emb),
        "wqkv": bf(Wqkv), "wo": bf(Wo), "w1": bf(W1), "w2": bf(W2),
        "ln1s": lnp(ln1_s), "ln1b": lnp(ln1_b),
        "ln2s": lnp(ln2_s), "ln2b": lnp(ln2_b),
        "ans": f32(an_s), "anb": f32(an_b),
        "top": bf(top),
        "topT": f32(top.T),
    }
    ar = np.arange(T)
    in_maps = []
    for b in range(B):
        pad = (ar < int(xs_lens[b]))
        mk = np.asarray(mask_indices[b], bool)
        m = {
            "xsT": f32(np.asarray(xs[b], np.float32).T),
            "mask01": f32(np.broadcast_to(mk[None, :].astype(np.float32), (P, T))),
            "notmask": f32(np.broadcast_to((~mk)[None, :].astype(np.float32), (P, T))),
            "padb": f32(np.where(pad, 0.0, NEG).astype(np.float32)),
            "mval": f32((pad & mk).astype(np.float32)),
        }
        m.update(shared)
        in_maps.append(m)
    return in_maps


_NC_CACHE = {}


def get_nc():
    if "nc" not in _NC_CACHE:
        _NC_CACHE["nc"] = build_nc()
    return _NC_CACHE["nc"]


def kernel(**inputs):
    inputs = {k: np.asarray(v) for k, v in inputs.items()}
    in_maps = make_inputs(**inputs)
    den = float(sum(m["mval"].sum() for m in in_maps)) * NCB
    nc = get_nc()
    res = run_bass_kernel_spmd(nc, in_maps, list(range(8)), trace=False)
    num = sum(float(r["num"][0, 0]) for r in res.results)
    return np.float32(num / den)


# revision 20
# speedup vs baseline: 1.0556x; 1.0556x over previous
"""BestRQ loss kernel for 8 Trainium2 NeuronCores.

Sharding: data-parallel over batch B=8 (one batch element per core, no
collectives). Each core runs: embed -> codebook argmin (targets) ->
mask-fill -> 6-layer transformer encoder -> logits over V=8192 ->
log-softmax gather -> partial masked sum. Host divides by mask count.

Device layout: residual stream is kept transposed [D, T] with D on SBUF
partitions (d-major), so every weight matmul uses weights in their natural
[din, dout] layout as lhsT. Attention scores are computed kpos-major so the
padding bias is a per-partition scalar folded into the ACT exp. V carries an
appended ones-column so the softmax denominators fall out of the AV matmul.
"""

import contextlib

import numpy as np
import ml_dtypes

import concourse.bass as bass
import concourse.tile as tile
from concourse import mybir
from concourse.bass_utils import run_bass_kernel_spmd
from concourse.masks import make_identity

F32 = mybir.dt.float32
BF16 = mybir.dt.bfloat16
AX = mybir.AxisListType
OP = mybir.AluOpType
AF = mybir.ActivationFunctionType

D = 512
T = 1024
V = 8192
NCB = 1
L = 6
H = 8
HD = 64
F = 2048
P = 128
KT = D // P       # 4 k-tiles over D
TT = T // P       # 8 token tiles
VC = V // 512     # 16 v chunks
EPS = 1e-5
NEG = -10000.0    # pad bias: exp(x*0.125 + NEG) == 0.0 exactly in fp32


def _legalize_single_wait(nc):
    """walrus in this container supports one sync-wait per instruction;
    split Tile's multi-wait tail drain into single-wait NOPs."""
    ctr = 0
    for fn in nc.m.functions:
        for bb in fn.blocks:
            insts = list(bb.instructions)
            out = []
            changed = False
            for inst in insts:
                si = getattr(inst, "sync_info", None)
                if si is not None and len(si.on_wait) > 1:
                    changed = True
                    waits = list(si.on_wait)
                    for w in waits[:-1]:
                        ctr += 1
                        nop = mybir.InstNoOp(name=f"{inst.name}-sw{ctr}", ins=[], outs=[])
                        nop.engine = inst.engine
                        nop.sync_info = mybir.SyncInfo(on_wait=[w], on_update=[])
                        out.append(nop)
                    inst.sync_info = mybir.SyncInfo(
                        on_wait=[waits[-1]], on_update=list(si.on_update)
                    )
                out.append(inst)
            if changed:
                bb.instructions = out
    return ctr


def _dram(nc, name, shape, dtype, out=False):
    return nc.declare_dram_parameter(name, list(shape), dtype, isOutput=out)


def _standardize(nc, sbuf, rows, ps_stat, r, ones_bt, sq_tag):
    """Per-token standardize of d-major tile r [128, KT, T] (fp32).
    Returns (rstd, negmrs) [128, T] fp32 partition-replicated tiles."""
    sq = sbuf.tile([P, KT, T], BF16, tag=sq_tag + "sq", name="lnsq")
    nc.scalar.square(sq[:], r[:])
    rbf = sbuf.tile([P, KT, T], BF16, tag=sq_tag, name="lnrbf")
    nc.vector.tensor_copy(rbf[:], r[:])
    mean = rows.tile([P, T], F32, tag="lnrow", name="mean")
    var = rows.tile([P, T], F32, tag="lnrow", name="var")
    for ch in range(2):
        sl = bass.ts(ch, 512)
        sum_ps = ps_stat.tile([P, 512], F32, tag="ps", name="sum_ps")
        for k in range(KT):
            nc.tensor.matmul(sum_ps[:], ones_bt[:], rbf[:, k, sl],
                             start=(k == 0), stop=(k == KT - 1))
        nc.vector.tensor_scalar_mul(mean[:, sl], sum_ps[:], 1.0 / D)
        ssq_ps = ps_stat.tile([P, 512], F32, tag="ps", name="ssq_ps")
        for k in range(KT):
            nc.tensor.matmul(ssq_ps[:], ones_bt[:], sq[:, k, sl],
                             start=(k == 0), stop=(k == KT - 1))
        nc.vector.tensor_scalar_mul(var[:, sl], ssq_ps[:], 1.0 / D)
    # var = ssq/D - mean^2 + eps ; rstd = 1/sqrt(var) ; negmrs = -mean*rstd
    sd = rows.tile([P, T], F32, tag="lnrow", name="sd")
    nc.vector.tensor_tensor(sd[:], mean[:], mean[:], OP.mult)
    nc.vector.tensor_tensor(var[:], var[:], sd[:], OP.subtract)
    nc.vector.tensor_scalar_add(var[:], var[:], float(EPS))
    nc.scalar.activation(sd[:], var[:], AF.Sqrt)
    rstd = rows.tile([P, T], F32, tag="lnrow", name="rstd")
    nc.vector.reciprocal(rstd[:], sd[:])
    nc.vector.tensor_tensor(mean[:], mean[:], rstd[:], OP.mult)
    nc.vector.tensor_scalar_mul(mean[:], mean[:], -1.0)
    return rstd, mean


def _ln_to(nc, sbuf, rows, ps_stat, r, ones_bt, s_t, b_t, out_tile, sq_tag, tmp_tag):
    """out_tile[:, k] = standardize(r)*s + b  (per-ko scale/bias), any out dtype."""
    rstd, negmrs = _standardize(nc, sbuf, rows, ps_stat, r, ones_bt, sq_tag)
    u = sbuf.tile([P, KT, T], F32, tag=tmp_tag, name="lntmp")
    for k in range(KT):
        nc.vector.tensor_tensor(u[:, k], r[:, k], rstd[:], OP.mult)
        nc.vector.tensor_tensor(u[:, k], u[:, k], negmrs[:], OP.add)
        nc.vector.tensor_scalar(out_tile[:, k], u[:, k], s_t[:, k : k + 1],
                                b_t[:, k : k + 1], OP.mult, OP.add)


def build_nc(cfg=()):
    cfg = set(cfg)
    nc = bass.Bass()

    # ---- per-core inputs (d-major / natural weight layouts) ----
    xsT_d = _dram(nc, "xsT", [D, T], F32)
    peT_d = _dram(nc, "peT", [D, T], F32)
    wemb_d = _dram(nc, "wemb", [D, D], F32)
    proj_d = _dram(nc, "proj", [D, D], F32)
    embh_d = _dram(nc, "embh", [D, V], BF16)
    embl_d = _dram(nc, "embl", [D, V], BF16)
    hesq_d = _dram(nc, "hesq", [P, V], F32)
    ilns_d = _dram(nc, "ilns", [D], F32)
    ilnb_d = _dram(nc, "ilnb", [D], F32)
    memb_d = _dram(nc, "memb", [D], F32)
    mask01_d = _dram(nc, "mask01", [P, T], F32)
    notmask_d = _dram(nc, "notmask", [P, T], F32)
    padb_d = _dram(nc, "padb", [T], F32)
    mval_d = _dram(nc, "mval", [T], F32)
    wqkv_d = _dram(nc, "wqkv", [L, D, 3 * D], BF16)
    wo_d = _dram(nc, "wo", [L, D, D], BF16)
    w1_d = _dram(nc, "w1", [L, D, F], BF16)
    w2_d = _dram(nc, "w2", [L, F, D], BF16)
    ln1s_d = _dram(nc, "ln1s", [P, KT, L], F32)
    ln1b_d = _dram(nc, "ln1b", [P, KT, L], F32)
    ln2s_d = _dram(nc, "ln2s", [P, KT, L], F32)
    ln2b_d = _dram(nc, "ln2b", [P, KT, L], F32)
    ans_d = _dram(nc, "ans", [D], F32)
    anb_d = _dram(nc, "anb", [D], F32)
    top_d = _dram(nc, "top", [D, V], BF16)
    topT_d = _dram(nc, "topT", [V, D], BF16)

    num_d = _dram(nc, "num", [1, 1], F32, out=True)

    kp = lambda ap: ap.rearrange("(ko p) -> p ko", p=P)       # [D] -> [128, KT]
    dm = lambda ap: ap.rearrange("(ko p) t -> p ko t", p=P)   # [D,T] -> [128,KT,T]

    stack = contextlib.ExitStack()
    with tile.TileContext(nc) as tc, stack:
        persist = stack.enter_context(tc.tile_pool(name="persist", bufs=1))
        rows = stack.enter_context(tc.tile_pool(name="rows", bufs=4))
        tiny = stack.enter_context(tc.tile_pool(name="tiny", bufs=2))
        ps_mm = stack.enter_context(tc.tile_pool(name="ps_mm", bufs=6, space="PSUM"))
        ps_stat = ps_mm

        ones_t = persist.tile([P, P], F32)
        nc.vector.memset(ones_t[:], 1.0)
        ones_bt = persist.tile([P, P], BF16)
        nc.vector.memset(ones_bt[:], 1.0)
        ident = persist.tile([P, P], F32)
        make_identity(nc, ident[:])
        r_t = persist.tile([P, KT, T], F32)          # residual stream (masked)
        tgt_t = persist.tile([P, TT], mybir.dt.int32)
        nc.vector.memset(tgt_t[:], 0)
        tlogit_t = persist.tile([P, TT], F32)
        lse_t = persist.tile([P, TT], F32)
        padb_t = persist.tile([P, TT], F32)
        nc.sync.dma_start(out=padb_t[:], in_=padb_d.rearrange("(tt p) -> p tt", p=P))
        mval_t = persist.tile([P, TT], F32)
        nc.sync.dma_start(out=mval_t[:], in_=mval_d.rearrange("(tt p) -> p tt", p=P))

        # ================= stage A/B: embed, targets =================
        with tc.tile_pool(name="pre", bufs=1) as pre:
            z_t = pre.tile([P, KT, T], F32)
            with tc.tile_pool(name="preA", bufs=1) as preA:
                x_t = preA.tile([P, KT, T], F32)
                ui_t = preA.tile([P, KT, T], F32)
                xsT_t = preA.tile([P, KT, T], F32)
                nc.sync.dma_start(out=xsT_t[:], in_=dm(xsT_d))
                peT_t = preA.tile([P, KT, T], F32)
                nc.sync.dma_start(out=peT_t[:], in_=dm(peT_d))
                wemb_t = preA.tile([P, KT, D], F32)
                nc.sync.dma_start(out=wemb_t[:], in_=dm(wemb_d))
                for mo in range(KT):
                    for ch in range(2):
                        sl = bass.ts(ch, 512)
                        ps = ps_mm.tile([P, 512], F32, tag="ps", name="ps")
                        for k in range(KT):
                            nc.tensor.matmul(ps[:], wemb_t[:, k, bass.ts(mo, P)],
                                             xsT_t[:, k, sl],
                                             start=(k == 0), stop=(k == KT - 1))
                        nc.vector.tensor_tensor(x_t[:, mo, sl], ps[:], peT_t[:, mo, sl], OP.add)

                # masked residual r = x*notmask + mask_emb*mask (encoder input)
                mask01_t = preA.tile([P, T], F32)
                nc.sync.dma_start(out=mask01_t[:], in_=mask01_d[:])
                notmask_t = preA.tile([P, T], F32)
                nc.sync.dma_start(out=notmask_t[:], in_=notmask_d[:])
                memb_t = preA.tile([P, KT], F32)
                nc.sync.dma_start(out=memb_t[:], in_=kp(memb_d))
                for k in range(KT):
                    nc.vector.tensor_tensor(r_t[:, k], x_t[:, k], notmask_t[:], OP.mult)
                    nc.vector.scalar_tensor_tensor(
                        r_t[:, k], mask01_t[:],
                        memb_t[:, k : k + 1], r_t[:, k], OP.mult, OP.add)

                # iln -> ui
                ilns_t = preA.tile([P, KT], F32)
                nc.sync.dma_start(out=ilns_t[:], in_=kp(ilns_d))
                ilnb_t = preA.tile([P, KT], F32)
                nc.sync.dma_start(out=ilnb_t[:], in_=kp(ilnb_d))
                _ln_to(nc, preA, rows, ps_stat, x_t, ones_bt, ilns_t, ilnb_t, ui_t,
                       "lnscr_pre", "lnscr_pre")

                # projection -> z (fp32)
                proj_t = preA.tile([P, KT, D], F32)
                nc.sync.dma_start(out=proj_t[:], in_=dm(proj_d))
                for mo in range(KT):
                    for ch in range(2):
                        sl = bass.ts(ch, 512)
                        ps = ps_mm.tile([P, 512], F32, tag="ps", name="ps")
                        for k in range(KT):
                            nc.tensor.matmul(ps[:], proj_t[:, k, bass.ts(mo, P)],
                                             ui_t[:, k, sl],
                                             start=(k == 0), stop=(k == KT - 1))
                        nc.vector.tensor_copy(z_t[:, mo, sl], ps[:])

            # cdist scores + argmax (bf16x3), token tiles in groups of 2
            with tc.tile_pool(name="cd", bufs=1) as cd, \
                 tc.tile_pool(name="cd2", bufs=2) as cd2:
                hesq_t = cd.tile([P, V], F32)
                nc.sync.dma_start(out=hesq_t[:], in_=hesq_d[:])
                zh_t = cd.tile([P, KT, T], BF16)
                nc.vector.tensor_copy(zh_t[:], z_t[:])
                zlf_t = cd.tile([P, KT, T], F32)
                nc.vector.tensor_tensor(zlf_t[:], z_t[:], zh_t[:], OP.subtract)
                zl_t = cd.tile([P, KT, T], BF16)
                nc.vector.tensor_copy(zl_t[:], zlf_t[:])
                for g in range(0 if 'no_cdist' in cfg else 4):
                    sc_tiles = []
                    for i in range(2):
                        sc_tiles.append(cd.tile([P, V], F32, tag=f"sc{i}", name=f"sc{i}"))
                    for vc in range(VC):
                        vsl = bass.ts(vc, 512)
                        eh_t = cd2.tile([P, KT, 512], BF16, tag="ehchunk", name="ehchunk")
                        nc.sync.dma_start(out=eh_t[:], in_=dm(embh_d)[:, :, vsl])
                        el_t = cd2.tile([P, KT, 512], BF16, tag="elchunk", name="elchunk")
                        nc.sync.dma_start(out=el_t[:], in_=dm(embl_d)[:, :, vsl])
                        for i in range(2):
                            tt = g * 2 + i
                            tsl = bass.ts(tt, P)
                            ps = ps_mm.tile([P, 512], F32, tag="ps", name="ps")
                            for k in range(KT):
                                nc.tensor.matmul(ps[:], zh_t[:, k, tsl], eh_t[:, k],
                                                 start=(k == 0), stop=False)
                            for k in range(KT):
                                nc.tensor.matmul(ps[:], zh_t[:, k, tsl], el_t[:, k],
                                                 start=False, stop=False)
                            for k in range(KT):
                                nc.tensor.matmul(ps[:], zl_t[:, k, tsl], eh_t[:, k],
                                                 start=False, stop=(k == KT - 1))
                            nc.vector.tensor_tensor(
                                sc_tiles[i][:, vsl], ps[:],
                                hesq_t[:, vsl], OP.subtract)
                    for i in range(2):
                        tt = g * 2 + i
                        mv = tiny.tile([P, 8], F32, tag="mv", name="mv")
                        mi = tiny.tile([P, 8], mybir.dt.uint32, tag="mi", name="mi")
                        nc.vector.max(mv[:], sc_tiles[i][:])
                        nc.vector.max_index(mi[:], mv[:], sc_tiles[i][:])
                        nc.vector.tensor_copy(tgt_t[:, tt : tt + 1], mi[:, 0:1])

        # ================= stage D: encoder =================
        enc_stack = contextlib.ExitStack()
        enc = enc_stack.enter_context(tc.tile_pool(name="enc", bufs=1))
        encw = enc_stack.enter_context(tc.tile_pool(name="encw", bufs=1))
        encw1 = enc_stack.enter_context(tc.tile_pool(name="encw1", bufs=1))
        ps_av = enc_stack.enter_context(tc.tile_pool(name="ps_av", bufs=2, space="PSUM"))

        ln1s_t = enc.tile([P, KT, L], F32)
        nc.sync.dma_start(out=ln1s_t[:], in_=ln1s_d[:])
        ln1b_t = enc.tile([P, KT, L], F32)
        nc.sync.dma_start(out=ln1b_t[:], in_=ln1b_d[:])
        ln2s_t = enc.tile([P, KT, L], F32)
        nc.sync.dma_start(out=ln2s_t[:], in_=ln2s_d[:])
        ln2b_t = enc.tile([P, KT, L], F32)
        nc.sync.dma_start(out=ln2b_t[:], in_=ln2b_d[:])

        for l in range(0 if 'no_enc' in cfg else L):
            wqkv_t = encw.tile([P, KT, 3 * D], BF16, tag="wqkv", name="wqkv")
            nc.sync.dma_start(out=wqkv_t[:],
                              in_=wqkv_d[l].rearrange("(ko p) m -> p ko m", p=P))
            # LN1 -> u (bf16)
            u_t = enc.tile([P, KT, T], BF16, tag="u", name="u")
            _ln_to(nc, enc, rows, ps_stat, r_t, ones_bt,
                   ln1s_t[:, :, l], ln1b_t[:, :, l], u_t, "lnscr", "lnscr")

            # q,k dims-major: qk [128, 8mo, T] bf16
            qk_t = enc.tile([P, 8, T], BF16, tag="qk", name="qk")
            if 'no_qkv' in cfg:
                nc.vector.memset(qk_t[:], 0)
            for mo in range(0 if 'no_qkv' in cfg else 8):
                for ch in range(2):
                    sl = bass.ts(ch, 512)
                    ps = ps_mm.tile([P, 512], F32, tag="ps", name="ps")
                    for k in range(KT):
                        nc.tensor.matmul(ps[:], wqkv_t[:, k, bass.ts(mo, P)],
                                         u_t[:, k, sl],
                                         start=(k == 0), stop=(k == KT - 1))
                    nc.vector.tensor_copy(qk_t[:, mo, sl], ps[:])

            # v token-major with appended ones column: v [128, tt, h, 65]
            v_t = enc.tile([P, TT, H, 2 * HD], BF16, tag="v", name="v")
            nc.vector.memset(v_t[:, :, :, HD : 2 * HD], 1.0)
            for tt in range(TT):
                ps = ps_mm.tile([P, 512], F32, tag="ps", name="ps")
                for k in range(KT):
                    nc.tensor.matmul(ps[:], u_t[:, k, bass.ts(tt, P)],
                                     wqkv_t[:, k, 2 * D : 3 * D],
                                     start=(k == 0), stop=(k == KT - 1))
                nc.vector.tensor_copy(
                    v_t[:, tt, :, 0:HD],
                    ps[:].rearrange("p (h d) -> p h d", h=H))

            # attention, (head, q-half) tiles; attnT [128, KT(hd-major), T] bf16
            attnT_t = enc.tile([P, KT, T], BF16, tag="attnT", name="attnT")
            if 'no_attn' in cfg:
                nc.vector.memset(attnT_t[:], 0)
            for h in range(0 if 'no_attn' in cfg else H):
                bp = 64 * (h % 2)
                qmo, kmo = h // 2, 4 + h // 2
                for ch in range(2):
                    sl = bass.ts(ch, 512)
                    a_t = enc.tile([P, TT, 512], BF16, tag="a", name="a", bufs=2)
                    for kt in range(TT):
                        ps = ps_mm.tile([P, 512], F32, tag="ps", name="ps")
                        nc.tensor.matmul(
                            ps[:], qk_t[bp : bp + HD, kmo, bass.ts(kt, P)],
                            qk_t[bp : bp + HD, qmo, sl], start=True, stop=True)
                        nc.scalar.activation(a_t[:, kt, :], ps[:], AF.Exp,
                                             bias=padb_t[:, kt : kt + 1], scale=0.125)
                    pso = ps_av.tile([P, 512], F32, tag="ps_av", name="ps_av")
                    for kt in range(TT):
                        nc.tensor.matmul(pso[:], v_t[:, kt, h, :], a_t[:, kt, :],
                                         start=(kt == 0), stop=(kt == TT - 1))
                    rec = tiny.tile([HD, 512], F32, tag="rec", name="rec")
                    nc.vector.reciprocal(rec[:], pso[HD : 2 * HD, :])
                    nc.vector.tensor_tensor(
                        attnT_t[bp : bp + HD, h // 2, sl], pso[0:HD, :],
                        rec[:], OP.mult)

            # Wo + residual
            wo_t = encw1.tile([P, KT, D], BF16, tag="wo", name="wo")
            nc.sync.dma_start(out=wo_t[:], in_=wo_d[l].rearrange("(ko p) m -> p ko m", p=P))
            for mo in range(KT):
                for ch in range(2):
                    sl = bass.ts(ch, 512)
                    ps = ps_mm.tile([P, 512], F32, tag="ps", name="ps")
                    for k in range(KT):
                        nc.tensor.matmul(ps[:], wo_t[:, k, bass.ts(mo, P)],
                                         attnT_t[:, k, sl],
                                         start=(k == 0), stop=(k == KT - 1))
                    nc.vector.tensor_tensor(r_t[:, mo, sl], r_t[:, mo, sl], ps[:], OP.add)

            # LN2 -> u2 (bf16), FFN
            u2_t = enc.tile([P, KT, T], BF16, tag="u", name="u2")
            _ln_to(nc, enc, rows, ps_stat, r_t, ones_bt,
                   ln2s_t[:, :, l], ln2b_t[:, :, l], u2_t, "lnscr", "lnscr")
            w1_t = encw1.tile([P, KT, F], BF16, tag="w1", name="w1")
            nc.sync.dma_start(out=w1_t[:], in_=w1_d[l].rearrange("(ko p) m -> p ko m", p=P))
            f_t = enc.tile([P, F // P, T], BF16, tag="f", name="f")
            if 'no_ffn' in cfg:
                nc.vector.memset(f_t[:], 0)
            for mo in range(0 if 'no_ffn' in cfg else F // P):
                for ch in range(2):
                    sl = bass.ts(ch, 512)
                    ps = ps_mm.tile([P, 512], F32, tag="ps", name="ps")
                    for k in range(KT):
                        nc.tensor.matmul(ps[:], w1_t[:, k, bass.ts(mo, P)],
                                         u2_t[:, k, sl],
                                         start=(k == 0), stop=(k == KT - 1))
                    nc.scalar.activation(f_t[:, mo, sl], ps[:], AF.Relu)
            w2_t = encw1.tile([P, F // P, D], BF16, tag="w2", name="w2")
            if False:
                pass
            nc.sync.dma_start(out=w2_t[:], in_=w2_d[l].rearrange("(ko p) m -> p ko m", p=P))
            for mo in range(KT):
                for ch in range(2):
                    sl = bass.ts(ch, 512)
                    ps = ps_mm.tile([P, 512], F32, tag="ps", name="ps")
                    for k in range(F // P):
                        nc.tensor.matmul(ps[:], w2_t[:, k, bass.ts(mo, P)],
                                         f_t[:, k, sl],
                                         start=(k == 0), stop=(k == F // P - 1))
                    nc.vector.tensor_tensor(r_t[:, mo, sl], r_t[:, mo, sl], ps[:], OP.add)

        enc_stack.close()

        # ================= stage E: final LN, logits, loss =================
        with tc.tile_pool(name="fin", bufs=1) as fin, \
             tc.tile_pool(name="fin2", bufs=3) as fin2:
            ans_t = fin.tile([P, KT], F32)
            nc.sync.dma_start(out=ans_t[:], in_=kp(ans_d))
            anb_t = fin.tile([P, KT], F32)
            nc.sync.dma_start(out=anb_t[:], in_=kp(anb_d))
            uan_t = fin.tile([P, KT, T], F32)
            _ln_to(nc, fin, rows, ps_stat, r_t, ones_bt, ans_t, anb_t, uan_t,
                   "lnscr_fin", "lnscr_fin")
            uanb_t = fin.tile([P, KT, T], BF16)
            nc.vector.tensor_copy(uanb_t[:], uan_t[:])

            # gather target rows of topT, dot with u_an (token-major transpose)
            for tt in range(TT):
                utok = fin2.tile([P, D], F32, tag="utok", name="utok")
                for k in range(KT):
                    pst = ps_mm.tile([P, 512], F32, tag="ps", name="pst")
                    nc.tensor.transpose(pst[:, 0:P], uan_t[:, k, bass.ts(tt, P)], ident[:])
                    nc.vector.tensor_copy(utok[:, bass.ts(k, P)], pst[:, 0:P])
                g_t = fin2.tile([P, D], BF16, tag="g", name="g")
                nc.gpsimd.indirect_dma_start(
                    out=g_t[:], out_offset=None, in_=topT_d[:],
                    in_offset=bass.IndirectOffsetOnAxis(ap=tgt_t[:, tt : tt + 1], axis=0))
                prod = fin2.tile([P, D], F32, tag="prod", name="prod")
                nc.vector.tensor_tensor(prod[:], g_t[:], utok[:], OP.mult)
                nc.vector.reduce_sum(tlogit_t[:, tt : tt + 1], prod[:], axis=AX.X)

            # logits + lse
            acc_t = fin.tile([P, TT, VC], F32)
            nc.vector.memset(acc_t[:], 1.0)
            for vc in range(0 if 'no_logits' in cfg else VC):
                vsl = bass.ts(vc, 512)
                top_t = fin2.tile([P, KT, 512], BF16, tag="topchunk", name="topchunk")
                nc.sync.dma_start(out=top_t[:], in_=dm(top_d)[:, :, vsl])
                for tt in range(TT):
                    ps = ps_mm.tile([P, 512], F32, tag="ps", name="ps")
                    for k in range(KT):
                        nc.tensor.matmul(ps[:], uanb_t[:, k, bass.ts(tt, P)],
                                         top_t[:, k],
                                         start=(k == 0), stop=(k == KT - 1))
                    esc = fin2.tile([P, 512], BF16, tag="esc", name="esc")
                    nc.scalar.activation(esc[:], ps[:], AF.Exp,
                                         accum_out=acc_t[:, tt, vc : vc + 1])
            sume = fin.tile([P, TT], F32)
            nc.vector.reduce_sum(sume[:], acc_t[:], axis=AX.X)
            nc.scalar.activation(lse_t[:], sume[:], AF.Ln)

            # num = sum((lse - tlogit) * mval)
            ent = fin.tile([P, TT], F32)
            nc.vector.tensor_tensor(ent[:], lse_t[:], tlogit_t[:], OP.subtract)
            nc.vector.tensor_tensor(ent[:], ent[:], mval_t[:], OP.mult)
            entsum = fin.tile([P, 1], F32)
            nc.vector.reduce_sum(entsum[:], ent[:], axis=AX.X)
            nps = ps_stat.tile([1, 1], F32, tag="ps", name="nps")
            nc.tensor.matmul(nps[:], ones_t[:, 0:1], entsum[:], start=True, stop=True)
            nout = fin.tile([1, 1], F32)
            nc.vector.tensor_copy(nout[:], nps[:])
            nc.sync.dma_start(out=num_d[:], in_=nout[:])

    _legalize_single_wait(nc)
    return nc


def make_inputs(xs, xs_lens, mask_indices, W_embed, ln1_s, ln1_b, Wqkv, Wo,
                ln2_s, ln2_b, W1, W2, an_s, an_b, iln_s, iln_b, mask_emb,
                top_n_out, projection, embeddings):
    """Host-side prep: transposes, casts, masks, PE table. Returns in_maps."""
    B = xs.shape[0]
    bf = lambda a: np.ascontiguousarray(a).astype(ml_dtypes.bfloat16)
    lnp = lambda a: np.ascontiguousarray(
        np.transpose(np.asarray(a, np.float32).reshape(L, KT, P), (2, 1, 0)))
    f32 = lambda a: np.ascontiguousarray(np.asarray(a, np.float32))

    pos = np.arange(T, dtype=np.float32)[:, None]
    i = np.arange(0, D, 2, dtype=np.float32)[None, :]
    ang = pos / np.power(10000.0, i / D)
    pe = np.zeros((T, D), np.float32)
    pe[:, 0::2] = np.sin(ang)
    pe[:, 1::2] = np.cos(ang)

    emb = np.asarray(embeddings[0], np.float32)          # [E, V]
    emb_hi = emb.astype(ml_dtypes.bfloat16)
    emb_lo = (emb - emb_hi.astype(np.float32)).astype(ml_dtypes.bfloat16)
    hesq = 0.5 * (emb.astype(np.float64) ** 2).sum(0).astype(np.float32)[None, :]
    top = np.asarray(top_n_out[0], np.float32)           # [D, V]

    shared = {
        "peT": f32(pe.T),
        "wemb": f32(W_embed),
        "proj": f32(projection),
        "embh": emb_hi, "embl": emb_lo,
        "hesq": f32(np.broadcast_to(hesq, (P, V))),
        "ilns": f32(iln_s), "ilnb": f32(iln_b),
        "memb": f32(mask_emb),
        "wqkv": bf(Wqkv), "wo": bf(Wo), "w1": bf(W1), "w2": bf(W2),
        "ln1s": lnp(ln1_s), "ln1b": lnp(ln1_b),
        "ln2s": lnp(ln2_s), "ln2b": lnp(ln2_b),
        "ans": f32(an_s), "anb": f32(an_b),
        "top": bf(top),
        "topT": bf(top.T),
    }
    ar = np.arange(T)
    in_maps = []
    for b in range(B):
        pad = (ar < int(xs_lens[b]))
        mk = np.asarray(mask_indices[b], bool)
        m = {
            "xsT": f32(np.asarray(xs[b], np.float32).T),
            "mask01": f32(np.broadcast_to(mk[None, :].astype(np.float32), (P, T))),
            "notmask": f32(np.broadcast_to((~mk)[None, :].astype(np.float32), (P, T))),
            "padb": f32(np.where(pad, 0.0, NEG).astype(np.float32)),
            "mval": f32((pad & mk).astype(np.float32)),
        }
        m.update(shared)
        in_maps.append(m)
    return in_maps


_NC_CACHE = {}


def get_nc():
    if "nc" not in _NC_CACHE:
        _NC_CACHE["nc"] = build_nc()
    return _NC_CACHE["nc"]


def kernel(**inputs):
    inputs = {k: np.asarray(v) for k, v in inputs.items()}
    in_maps = make_inputs(**inputs)
    den = float(sum(m["mval"].sum() for m in in_maps)) * NCB
    nc = get_nc()
    res = run_bass_kernel_spmd(nc, in_maps, list(range(8)), trace=False)
    num = sum(float(r["num"][0, 0]) for r in res.results)
    return np.float32(num / den)
